# revision 42
# baseline (speedup 1.0000x reference)
"""CMHSA (conv-multi-head-self-attention) Trainium2 kernel.

Full inputs -> full output. Internally shards across 8 NeuronCores:
core i handles batch i//4 and query columns [(i%4)*1024, (i%4+1)*1024)
of the flattened spatial dim N = H*W = 4096 (query sharding: each core
computes K/V for all 8 heads of its batch, attention + output projection
for its own query columns; host gather is a pure concat).

Layout notes (per core, all matmuls in f32r = fp32 data streamed at
full PE rate, ~1.5e-4 component accuracy):
  - k_rep[h]: [128, 4096] = head h's K (32 rows) replicated 4x across
    partition groups, produced directly by projecting with column-
    replicated weights (same matmul cost; enables 4-way row-packed
    K=32 S^T matmuls via tile_position).
  - S^T tiles [m-tile 128, q 512] so the softmax sum runs over
    partitions via a ones-column in V^T (M=33 AV matmul); exp on
    ScalarE with the 1/sqrt(d) scale folded in; no max-subtraction
    (logits are O(1) for this problem's scales).
  - normalization: 1/Z via DVE reciprocal, broadcast across 32
    partitions with a K=1 fp32 matmul, applied with one DVE multiply.
  - projections for head h+1 are emitted as small run-once thunks
    interleaved into head h's attention groups (with ensure-before-use
    maps) so PE/DVE work stays spread out; AV matmuls lag their exp by
    one group and the last group + normalization carry into the next
    chunk, so the in-order PE queue never stalls on the last exp of a
    chunk at chunk/head boundaries.

TimelineSim (calibrated cost model): ~323us per core; ScalarE exp is
the bottleneck engine (~252us busy; 33.6M softmax exps per core at
1 elem/cycle/lane). HW-verified absmax error vs the fp32 reference:
2.3e-6 (4.6e-7 of output absmax).
"""

import os
import sys

if '/opt/trn_rl_repo' not in sys.path:
    sys.path.insert(0, '/opt/trn_rl_repo')

import numpy as np
import ml_dtypes

B, C, HH, WW = 2, 256, 64, 64
N = HH * WW            # 4096
NHEADS = 8
D = C // NHEADS        # 32
NCORES = 8
QSHARD = 4             # query shards per batch
NQ = N // QSHARD       # 1024 queries per core
CT = C // 128          # 2 c-tiles
NT = N // 128          # 32 m/n-tiles
SCALE = float(D) ** -0.5

_CACHE = {}


def _build():
    import concourse.bacc as bacc
    import concourse.mybir as mybir
    import concourse.tile as tile

    F32 = mybir.dt.float32
    F32R = mybir.dt.float32r
    BF16 = mybir.dt.bfloat16
    Exp = mybir.ActivationFunctionType.Exp

    dbg = os.environ.get("BASSDBG", "0") == "1"

    nc = bacc.Bacc("TRN2", target_bir_lowering=False, debug=False,
                   num_devices=NCORES)

    x_d = nc.dram_tensor("x", [C, N], F32R, kind="ExternalInput").ap()
    xq_d = nc.dram_tensor("xq", [C, NQ], F32R, kind="ExternalInput").ap()
    xqf_d = nc.dram_tensor("xqf", [C, NQ], F32, kind="ExternalInput").ap()
    wqt_d = nc.dram_tensor("wqt", [C, 1024], F32R, kind="ExternalInput").ap()
    wkt_d = nc.dram_tensor("wkt", [C, 1024], F32R, kind="ExternalInput").ap()
    wvt_d = nc.dram_tensor("wvt", [C, C], F32R, kind="ExternalInput").ap()
    wot_d = nc.dram_tensor("wot", [C, C], F32R, kind="ExternalInput").ap()
    bias_d = nc.dram_tensor("bias", [128, 20], F32, kind="ExternalInput").ap()
    out_d = nc.dram_tensor("out", [C, NQ], F32, kind="ExternalOutput").ap()
    if dbg:
        dbg_krep = nc.dram_tensor("dbg_krep", [128, N], F32,
                                  kind="ExternalOutput").ap()
        dbg_qrep = nc.dram_tensor("dbg_qrep", [128, NQ], F32,
                                  kind="ExternalOutput").ap()
        dbg_vt = nc.dram_tensor("dbg_vt", [128, NHEADS * (D + 1)], F32,
                                kind="ExternalOutput").ap()
        dbg_av = nc.dram_tensor("dbg_av", [D + 1, 512], F32,
                                kind="ExternalOutput").ap()
        dbg_zr = nc.dram_tensor("dbg_zr", [1, 512], F32,
                                kind="ExternalOutput").ap()

    x_dr = x_d.rearrange("(t p) n -> p t n", p=128)
    xqf_dr = xqf_d.rearrange("(t p) n -> p t n", p=128)
    xq_dr = xq_d.rearrange("(t p) n -> p t n", p=128)
    wqt_dr = wqt_d.rearrange("(t p) m -> p t m", p=128)
    wkt_dr = wkt_d.rearrange("(t p) m -> p t m", p=128)
    wvt_dr = wvt_d.rearrange("(t p) m -> p t m", p=128)
    wot_dr = wot_d.rearrange("(t p) m -> p t m", p=128)
    out_dr = out_d.rearrange("(t p) n -> p t n", p=128)

    with tile.TileContext(nc) as tc:
        with tc.tile_pool(name="const", bufs=1) as cpool, \
             tc.tile_pool(name="kq", bufs=1) as kqpool, \
             tc.tile_pool(name="work", bufs=1) as wpool, \
             tc.tile_pool(name="ps", bufs=1, space="PSUM") as ps:

            # ---- loads, ordered so head-0 projections start ASAP ----
            wkt_r = cpool.tile([128, CT, 1024], F32R)
            wqt_r = cpool.tile([128, CT, 1024], F32R)
            xq_r = cpool.tile([128, CT, NQ], F32R)
            x_r = cpool.tile([128, CT, N], F32R)
            wvt_r = cpool.tile([128, CT, C], F32R)
            wot_r = cpool.tile([128, CT, C], F32R)

            def wslice(t, h):
                return t[:, :, h * 128:(h + 1) * 128]

            bias_s = cpool.tile([128, 20], F32)
            nc.sync.dma_start(bias_s, bias_d)
            bqr_s = bias_s[:, 0:8]
            bkr_s = bias_s[:, 8:16]
            bvp_s = bias_s[:, 16:18]
            bop_s = bias_s[:, 18:20]
            nc.sync.dma_start(wslice(wkt_r, 0), wslice(wkt_dr, 0))
            nc.sync.dma_start(wslice(wqt_r, 0), wslice(wqt_dr, 0))
            nc.sync.dma_start(x_r[:, :, 0:256], x_dr[:, :, 0:256])
            nc.sync.dma_start(x_r[:, :, 256:512], x_dr[:, :, 256:512])
            nc.sync.dma_start(wvt_r, wvt_dr)
            for c2 in range(2):
                nc.sync.dma_start(xq_r[:, :, c2 * 512:(c2 + 1) * 512],
                                  xq_dr[:, :, c2 * 512:(c2 + 1) * 512])
            for xc in range(1, 8):
                nc.sync.dma_start(x_r[:, :, xc * 512:(xc + 1) * 512],
                                  x_dr[:, :, xc * 512:(xc + 1) * 512])
                h2 = xc
                nc.sync.dma_start(wslice(wkt_r, h2), wslice(wkt_dr, h2))
                nc.sync.dma_start(wslice(wqt_r, h2), wslice(wqt_dr, h2))
            nc.sync.dma_start(wot_r, wot_dr)
            xq_f = cpool.tile([128, CT, NQ], F32)
            nc.sync.dma_start(xq_f, xqf_dr)
            ones1f = cpool.tile([1, 32], F32)
            nc.vector.memset(ones1f, 1.0)
            ones1 = cpool.tile([1, 32], F32R)
            nc.vector.tensor_copy(ones1, ones1f)
            onesf = cpool.tile([128, 1], F32)
            nc.vector.memset(onesf, 1.0)

            vT_aug = wpool.tile([128, NT, NHEADS, D + 1], F32R)
            attnout = wpool.tile([128, CT, NQ], F32R)
            k_reps = {}
            q_reps = {}

            # --- deferred-work queue: small thunks the attention loop
            # drains between m-groups to keep PE/DVE busy w/o bursts.
            # Data-producing thunks are also registered in ensure-maps and
            # run eagerly right before their first consumer if the drain
            # pacing hasn't reached them yet.
            pending = []
            carry = []

            def once(f):
                state = [False]

                def go():
                    if not state[0]:
                        state[0] = True
                        f()
                return go

            def project(h):
                k_rep = kqpool.tile([128, N], F32R, tag="k_rep", bufs=2,
                                    name=f"k_rep{h}")
                q_rep = kqpool.tile([128, NQ], F32R, tag="q_rep", bufs=2,
                                    name=f"q_rep{h}")
                k_reps[h] = k_rep
                q_reps[h] = q_rep

                def kchunk(c8):
                    def go():
                        ps_k = ps.tile([128, 512], F32, tag="misc", bufs=1,
                                       name=f"psk{h}_{c8}")
                        for ct in range(CT):
                            nc.tensor.matmul(
                                ps_k,
                                wkt_r[:, ct, h * 128:(h + 1) * 128],
                                x_r[:, ct, c8 * 512:(c8 + 1) * 512],
                                start=(ct == 0), stop=(ct == CT - 1))
                        nc.vector.tensor_scalar_add(
                            k_rep[:, c8 * 512:(c8 + 1) * 512], ps_k,
                            bkr_s[:, h:h + 1])
                    return go

                def qchunk(c2):
                    def go():
                        # head 0's first q trip borrows the (still idle)
                        # AV bank so it runs parallel to kchunk(0)
                        tg = "av" if (h == 0 and c2 == 0) else "misc"
                        ps_q = ps.tile([128, 512], F32, tag=tg, bufs=1,
                                       name=f"psq{h}_{c2}")
                        for ct in range(CT):
                            nc.tensor.matmul(
                                ps_q,
                                wqt_r[:, ct, h * 128:(h + 1) * 128],
                                xq_r[:, ct, c2 * 512:(c2 + 1) * 512],
                                start=(ct == 0), stop=(ct == CT - 1))
                        nc.vector.tensor_scalar_add(
                            q_rep[:, c2 * 512:(c2 + 1) * 512], ps_q,
                            bqr_s[:, h:h + 1])
                    return go

                ks = [once(kchunk(c8)) for c8 in range(N // 512)]
                qs_ = [once(qchunk(c2)) for c2 in range(NQ // 512)]
                for c8, t in enumerate(ks):
                    kthunks[(h, c8)] = t
                for c2, t in enumerate(qs_):
                    qthunks[(h, c2)] = t
                return ks + qs_

            def vchunk(nt0):
                # two n-tiles per PSUM trip to halve misc-bank round-trips
                def go():
                    tg = "av" if nt0 == 0 else "misc"
                    ps_v = ps.tile([128, 512], F32, tag=tg, bufs=1,
                                   name=f"psv{nt0}")
                    for k in range(2):
                        for ct in range(CT):
                            nc.tensor.matmul(
                                ps_v[:, k * C:(k + 1) * C],
                                x_r[:, ct,
                                    (nt0 + k) * 128:(nt0 + k + 1) * 128],
                                wvt_r[:, ct, :],
                                start=(ct == 0), stop=(ct == CT - 1))
                    nc.vector.tensor_copy(
                        vT_aug[:, nt0:nt0 + 2, :, 0:D],
                        ps_v.rearrange("p (k h d) -> p k h d", k=2,
                                       h=NHEADS))
                return go

            def oproj(c2):
                def go():
                    qs = slice(c2 * 512, (c2 + 1) * 512)
                    for ot in range(CT):
                        # last block: st banks are idle by then, use one so
                        # both ot blocks pipeline in parallel banks
                        tg = "st" if (ot == 1 and c2 == 1) else "misc"
                        ps_o = ps.tile([128, 512], F32, tag=tg,
                                       bufs=(2 if tg == "st" else 1),
                                       name=f"pso{ot}_{c2}")
                        for ct in range(CT):
                            nc.tensor.matmul(
                                ps_o,
                                wot_r[:, ct, ot * 128:(ot + 1) * 128],
                                attnout[:, ct, qs],
                                start=(ct == 0), stop=(ct == CT - 1))
                        o_sb = wpool.tile([128, 512], F32, tag="o_sb",
                                          bufs=4, name=f"osb{ot}_{c2}")
                        nc.vector.tensor_add(o_sb, ps_o, xq_f[:, ot, qs])
                        nc.vector.tensor_scalar_add(o_sb, o_sb,
                                                    bop_s[:, ot:ot + 1])
                        nc.sync.dma_start(out_dr[:, ot, qs], o_sb)
                return go

            vthunks = {}
            kthunks = {}
            qthunks = {}

            def ensure_v(nt):
                t = vthunks.pop(nt - nt % 2, None)
                if t is not None:
                    t()

            def drain(k):
                for _ in range(k):
                    if pending:
                        pending.pop(0)()

            def attention(h):
                j4 = h % 4
                ct_h = h // 4
                k_rep = k_reps.pop(h)
                q_rep = q_reps.pop(h)
                if dbg and h == 0:
                    nc.sync.dma_start(dbg_krep, k_rep.bitcast(F32))
                    nc.sync.dma_start(dbg_qrep, q_rep.bitcast(F32))
                for qc in range(NQ // 512):
                    qs = slice(qc * 512, (qc + 1) * 512)
                    qt = qthunks.pop((h, qc), None)
                    if qt is not None:
                        qt()
                    ps_av = ps.tile([D + 1, 512], F32, tag="av", bufs=1,
                                    name=f"psav{h}_{qc}")
                    av_lag = []   # (mt, g, p_t) waiting to be emitted

                    def flush_av(k=None, av_lag=av_lag, ps_av=ps_av, h=h):
                        n = len(av_lag) if k is None else k
                        for _ in range(n):
                            lmt, lg, lp = av_lag.pop(0)
                            for i in range(lg):
                                nc.tensor.matmul(
                                    ps_av,
                                    vT_aug[:, lmt + i, h, :],
                                    lp[:, i * 512:(i + 1) * 512],
                                    start=(lmt + i == 0),
                                    stop=(lmt + i == NT - 1))

                    mt = 0
                    first = True
                    while mt < NT:
                        g = min(3, NT - mt)
                        st = ps.tile([128, 3 * 512], F32, tag="st", bufs=2,
                                     name=f"st{h}_{qc}_{mt}")
                        for i in range(g):
                            kt = kthunks.pop((h, ((mt + i) * 128) // 512),
                                             None)
                            if kt is not None:
                                kt()
                            ro = ((mt + i) % 4) * 32
                            nc.tensor.matmul(
                                st[:, i * 512:(i + 1) * 512],
                                k_rep[ro:ro + 32,
                                      (mt + i) * 128:(mt + i + 1) * 128],
                                q_rep[ro:ro + 32, qs],
                                start=True, stop=True,
                                tile_position=(ro, 0))
                        for i in range(g):
                            ensure_v(mt + i)
                        p_t = wpool.tile([128, 3 * 512], F32R, tag="p_t",
                                         bufs=4, name=f"pt{h}_{qc}_{mt}")
                        nc.scalar.activation(p_t[:, 0:g * 512],
                                             st[:, 0:g * 512],
                                             Exp, scale=SCALE)
                        if first:
                            # cross-chunk carry: previous chunk's last AV
                            # group + its normalize land here, behind this
                            # chunk's first S^T/exp.
                            while carry:
                                carry.pop(0)()
                            first = False
                        else:
                            drain(1)
                        av_lag.append((mt, g, p_t))
                        if len(av_lag) > 1:
                            flush_av(len(av_lag) - 1)
                        mt += g

                    def tail(h=h, qc=qc, qs=qs, ps_av=ps_av, j4=j4,
                             ct_h=ct_h, flush_av=flush_av):
                        flush_av()
                        if dbg and h == 0 and qc == 0:
                            av_dump = wpool.tile([D + 1, 512], F32,
                                                 name="av_dump")
                            nc.vector.tensor_copy(av_dump, ps_av)
                            nc.sync.dma_start(dbg_av, av_dump)
                        zr = wpool.tile([1, 512], F32R, tag="zr", bufs=2,
                                        name=f"zr{h}_{qc}")
                        with nc.allow_low_precision(reason="1/Z in f32r"):
                            nc.vector.reciprocal(zr, ps_av[D:D + 1, :])
                        if dbg and h == 0 and qc == 0:
                            nc.sync.dma_start(dbg_zr, zr.bitcast(F32))
                        bc = ps.tile([32, 512], F32, tag="misc", bufs=1,
                                     name=f"bc{h}_{qc}")
                        nc.tensor.matmul(bc, ones1, zr, start=True,
                                         stop=True)
                        dst = attnout[j4 * 32:(j4 + 1) * 32, ct_h, qs]
                        nc.vector.tensor_copy(dst, ps_av[0:D, :])
                        nc.vector.tensor_mul(dst, dst, bc)
                        nc.vector.tensor_scalar_add(
                            dst, dst, bvp_s[j4 * 32:(j4 + 1) * 32,
                                            ct_h:ct_h + 1])
                        if h == NHEADS - 1:
                            pending.append(oproj(qc))
                    carry.append(tail)

            # ones-column of vT_aug (free-dim broadcast copy)
            nc.vector.tensor_copy(
                vT_aug[:, :, :, D],
                onesf.to_broadcast([128, NT, NHEADS]))

            # head 0: first k/q chunks eagerly, rest interleaved
            p0 = project(0)
            p0[0]()           # kchunk 0
            p0[8]()           # qchunk 0
            p0[9]()           # qchunk 1
            pending.extend(p0[1:8])
            vthunks.update({nt: vchunk(nt) for nt in range(0, NT, 2)})
            ensure_v(0)
            if dbg:
                nc.sync.dma_start(dbg_vt, vT_aug[:, 0, :, :].bitcast(F32))
            for h in range(NHEADS):
                if h + 1 < NHEADS:
                    pending.extend(project(h + 1))
                attention(h)
                if h == 0:
                    for nt in range(NT):
                        ensure_v(nt)
            while carry:
                carry.pop(0)()
            while pending:
                pending.pop(0)()

    nc.compile()
    return nc


def get_program():
    if "nc" not in _CACHE:
        _CACHE["nc"] = _build()
    return _CACHE["nc"]


def make_in_maps(x, Wq, bq, Wk, bk, Wv, bv, Wo, bo):
    x = np.ascontiguousarray(np.asarray(x, dtype=np.float32))
    xr = x.reshape(B, C, N)
    wq = np.asarray(Wq, np.float32)
    wk = np.asarray(Wk, np.float32)
    wv = np.asarray(Wv, np.float32)
    wo = np.asarray(Wo, np.float32)

    # per-head 4x replicated transposed q/k weights: [c, h*128 + j*32 + d]
    def rep_t(w):
        # w: [out_c, c] -> out [c, 1024]
        wt = w.T.reshape(C, NHEADS, D)            # [c, h, d]
        r = np.repeat(wt[:, :, None, :], 4, axis=2)  # [c, h, 4, d]
        return np.ascontiguousarray(r.reshape(C, NHEADS * 128))

    wqt = rep_t(wq)
    wkt = rep_t(wk)
    wvt = np.ascontiguousarray(wv.T)
    wot = np.ascontiguousarray(wo.T)
    bqr = (np.tile(np.asarray(bq, np.float32).reshape(NHEADS, D), (1, 4))
           .reshape(NHEADS, 128).T)                # [128, 8]
    bkr = (np.tile(np.asarray(bk, np.float32).reshape(NHEADS, D), (1, 4))
           .reshape(NHEADS, 128).T)
    bvp = np.asarray(bv, np.float32).reshape(CT, 128).T
    bop = np.asarray(bo, np.float32).reshape(CT, 128).T
    bias = np.ascontiguousarray(
        np.concatenate([bqr, bkr, bvp, bop], axis=1))  # [128, 20]

    in_maps = []
    for core in range(NCORES):
        b = core // QSHARD
        q0 = (core % QSHARD) * NQ
        in_maps.append({
            "x": np.ascontiguousarray(xr[b]),
            "xq": np.ascontiguousarray(xr[b][:, q0:q0 + NQ]),
            "xqf": np.ascontiguousarray(xr[b][:, q0:q0 + NQ]),
            "wqt": wqt, "wkt": wkt, "wvt": wvt, "wot": wot,
            "bias": bias,
        })
    return in_maps


def gather(results):
    out = np.empty((B, C, N), np.float32)
    for core in range(NCORES):
        b = core // QSHARD
        q0 = (core % QSHARD) * NQ
        out[b][:, q0:q0 + NQ] = results[core]["out"]
    return out.reshape(B, C, HH, WW)


def kernel(**inputs):
    from concourse.bass_utils import run_bass_kernel_spmd
    nc = get_program()
    in_maps = make_in_maps(**inputs)
    res = run_bass_kernel_spmd(nc, in_maps, list(range(NCORES)))
    return gather(res.results)
